# revision 1
# baseline (speedup 1.0000x reference)
"""Trainium2 Bass kernel for per-expert MLP (MoE experts, expert-parallel).

Computes out = relu(relu(x @ w1) @ w2) per expert.
  x:  [E=32, N=1024, D_IN=3072] f32
  w1: [E, D_IN, D_H=1024] f32
  w2: [E, D_H, D_OUT=256] f32
  out:[E, N, D_OUT] f32

Sharding: expert dim E=32 split across 8 cores (4 experts/core), no
communication. Host pre-casts to bf16 and pre-tiles layouts so every DMA is
a plain partition-major copy and no on-chip transposes are needed:

GEMM1 computes hiddenT (h on partitions) directly:
  hiddenT[h, n] = sum_d w1[d, h] * x[n, d]
  lhsT = w1 tile [d(128 part), h(128 cols)]   (stationary)
  rhs  = xT tile [d(128 part), n(512 free)]   (moving)
GEMM2 then has contraction dim h already on partitions:
  outT[o, n] = sum_h w2[h, o] * hiddenT[h, n]
  lhsT = w2 tile [h(128), o(128)], rhs = hiddenT tile [h(128), n(512)]
The output is stored transposed ([E, D_OUT, N]) for contiguous DMA and
un-transposed on the host during gather.

Measured (8x trn2 NeuronCores, bf16 matmul / fp32 PSUM): HW exec time
~381 us/core, MFU ~91%, rel L2 error vs fp32 reference ~3.4e-3.
Roofline: 27.9 GFLOP/core at 78.6 TF/s bf16 = 355 us compute-bound floor
(+ ~19 us fixed Tile preamble/epilogue + ~5 us DMA-bound first-expert ramp).
"""

import numpy as np
import ml_dtypes

E, N, D_IN, D_H, D_OUT = 32, 1024, 3072, 1024, 256
NCORES = 8
E_PER = E // NCORES  # 4 experts per core
P = 128
DT = D_IN // P  # 24 k-tiles for GEMM1
HT = D_H // P   # 8 h-tiles
NT = N // P     # 8 n-tiles
FD = 512        # matmul free dim (one PSUM bank of f32)
NCH = N // FD   # 2 n-chunks in GEMM1

_BF16 = ml_dtypes.bfloat16
_CACHE = {}


def _build_program():
    """Build + compile the per-core Bass program (same program on all cores)."""
    if "nc" in _CACHE:
        return _CACHE["nc"], _CACHE["names"]

    from contextlib import ExitStack

    import concourse.bass as bass
    import concourse.tile as tile
    from concourse import bacc, mybir

    bf16 = mybir.dt.bfloat16
    f32 = mybir.dt.float32

    nc = bacc.Bacc("TRN2", target_bir_lowering=False, debug=False,
                   enable_asserts=False)

    # Per-core DRAM I/O (host-prepped layouts, see kernel() below).
    x_d = nc.dram_tensor("xt", [E_PER, P, DT, N], bf16, kind="ExternalInput").ap()
    w1_d = nc.dram_tensor("w1t", [E_PER, HT, P, DT * P], bf16,
                          kind="ExternalInput").ap()
    w2_d = nc.dram_tensor("w2t", [E_PER, P, HT, D_OUT], bf16,
                          kind="ExternalInput").ap()
    # Output stored transposed ([o, n] per expert): GEMM2 computes psum
    # [o=128, n=512] tiles, and this layout makes the store DMA fully
    # contiguous per partition. The host un-transposes after gather.
    out_d = nc.dram_tensor("out", [E_PER, D_OUT, N], f32,
                           kind="ExternalOutput").ap()

    relu = mybir.ActivationFunctionType.Relu

    with tile.TileContext(nc) as tc, ExitStack() as ctx:
        xp = ctx.enter_context(tc.tile_pool(name="x", bufs=2))
        w1p = ctx.enter_context(tc.tile_pool(name="w1", bufs=4))
        w2p = ctx.enter_context(tc.tile_pool(name="w2", bufs=2))
        hp = ctx.enter_context(tc.tile_pool(name="hid", bufs=2))
        op = ctx.enter_context(tc.tile_pool(name="o", bufs=2))
        wmp = ctx.enter_context(tc.tile_pool(name="warm", bufs=1))
        ps1 = ctx.enter_context(tc.tile_pool(name="ps1", bufs=6, space="PSUM"))
        ps2 = ctx.enter_context(tc.tile_pool(name="ps2", bufs=2, space="PSUM"))

        # PE warm-up: dummy matmuls with no data deps fill the initial DMA
        # wait so the HAM clock-gate is at 8/8 (2.4 GHz) when real matmuls
        # start (the un-throttle needs ~3.4us of sustained PE activity).
        # One accumulation group: no per-matmul PSUM WAW serialization.
        NWARM = 18
        warm = wmp.tile([P, FD], bf16, tag="warm")
        nc.vector.memset(warm[:], 0.0)
        pw = ps2.tile([P, FD], f32, tag="ps2", name="pw")
        for i in range(NWARM):
            nc.tensor.matmul(pw[:], warm[:, 0:P], warm[:],
                             start=(i == 0), stop=(i == NWARM - 1))

        for e in range(E_PER):
            # Load order on the sync HWDGE ring (FIFO per engine): w1 h0 and
            # the first x d-tile ahead of everything so GEMM1 h0 can start
            # immediately; remaining w1 tiles + w2 after the x stream.
            w1_tiles = []
            x_sb = xp.tile([P, DT, N], bf16, tag="x")
            w1_sb = w1p.tile([P, DT * P], bf16, tag="w1")
            if e == 0:
                # Chunk the first w1 tile so GEMM1's first matmuls only wait
                # on a 256KB transfer, and pace x per d-tile: the whole e0
                # ramp is DMA-bandwidth-bound, so start compute ASAP.
                w1b_sb = w1p.tile([P, DT * P], bf16, tag="w1")
                nc.sync.dma_start(w1_sb[:, 0: 8 * P], w1_d[e, 0, :, 0: 8 * P])
                w1_tiles.append(w1_sb)
                nc.sync.dma_start(x_sb[:, 0, :], x_d[e, :, 0, :])
                nc.sync.dma_start(w1b_sb[:, 0: 8 * P], w1_d[e, 1, :, 0: 8 * P])
                nc.sync.dma_start(x_sb[:, 1, :], x_d[e, :, 1, :])
                nc.sync.dma_start(x_sb[:, 2, :], x_d[e, :, 2, :])
                nc.sync.dma_start(w1_sb[:, 8 * P: 16 * P],
                                  w1_d[e, 0, :, 8 * P: 16 * P])
                nc.sync.dma_start(w1b_sb[:, 8 * P: 16 * P],
                                  w1_d[e, 1, :, 8 * P: 16 * P])
                nc.sync.dma_start(x_sb[:, 3, :], x_d[e, :, 3, :])
                nc.sync.dma_start(x_sb[:, 4, :], x_d[e, :, 4, :])
                nc.sync.dma_start(w1_sb[:, 16 * P: DT * P],
                                  w1_d[e, 0, :, 16 * P: DT * P])
                nc.sync.dma_start(w1b_sb[:, 16 * P: DT * P],
                                  w1_d[e, 1, :, 16 * P: DT * P])
                w1_tiles.append(w1b_sb)
                for d in range(5, 8):
                    nc.sync.dma_start(x_sb[:, d, :], x_d[e, :, d, :])
                for d in range(8, DT):
                    nc.sync.dma_start(x_sb[:, d, :], x_d[e, :, d, :])
            else:
                # prefetched during previous expert: coarse chunks to limit
                # HWDGE sem-lane churn (8 lanes shared across all queues)
                nc.sync.dma_start(w1_sb[:], w1_d[e, 0])
                w1_tiles.append(w1_sb)
                nc.sync.dma_start(x_sb[:, 0, :], x_d[e, :, 0, :])
                w1_sb = w1p.tile([P, DT * P], bf16, tag="w1")
                nc.sync.dma_start(w1_sb[:], w1_d[e, 1])
                w1_tiles.append(w1_sb)
                for d in range(1, 5):
                    nc.sync.dma_start(x_sb[:, d, :], x_d[e, :, d, :])
                for i in range(5, DT, 4):
                    j = min(i + 4, DT)
                    nc.sync.dma_start(x_sb[:, i:j, :], x_d[e, :, i:j, :])
            for h in range(2, HT):
                w1_sb = w1p.tile([P, DT * P], bf16, tag="w1")
                nc.sync.dma_start(w1_sb[:], w1_d[e, h])
                w1_tiles.append(w1_sb)
            w2_sb = w2p.tile([P, HT, D_OUT], bf16, tag="w2")
            nc.sync.dma_start(w2_sb[:], w2_d[e])

            hid = hp.tile([P, HT, N], bf16, tag="hid")

            # GEMM1 + relu -> hiddenT (bf16). h0 and h1 are interleaved in
            # one d-pass: each arriving x d-tile feeds 4 matmuls, so the
            # DMA-paced first-expert ramp consumes x at ~arrival rate
            # instead of stalling h0 on the tail of the x stream.
            pa = [ps1.tile([P, FD], f32, tag="ps1", name=f"pa{i}")
                  for i in range(2)]
            pb = [ps1.tile([P, FD], f32, tag="ps1", name=f"pb{i}")
                  for i in range(2)]
            for d in range(DT):
                for hh in range(2):
                    lhsT = w1_tiles[hh][:, bass.ts(d, P)]
                    nc.tensor.matmul(pa[hh][:], lhsT, x_sb[:, d, 0:FD],
                                     start=(d == 0), stop=(d == DT - 1))
                    nc.tensor.matmul(pb[hh][:], lhsT, x_sb[:, d, FD:N],
                                     start=(d == 0), stop=(d == DT - 1))
            for hh in range(2):
                nc.scalar.activation(hid[:, hh, 0:FD], pa[hh][:], relu)
                nc.scalar.activation(hid[:, hh, FD:N], pb[hh][:], relu)
            for h in range(2, HT):
                w1_sb = w1_tiles[h]
                pa1 = ps1.tile([P, FD], f32, tag="ps1")
                pb1 = ps1.tile([P, FD], f32, tag="ps1")
                for d in range(DT):
                    lhsT = w1_sb[:, bass.ts(d, P)]
                    nc.tensor.matmul(pa1[:], lhsT, x_sb[:, d, 0:FD],
                                     start=(d == 0), stop=(d == DT - 1))
                    nc.tensor.matmul(pb1[:], lhsT, x_sb[:, d, FD:N],
                                     start=(d == 0), stop=(d == DT - 1))
                nc.scalar.activation(hid[:, h, 0:FD], pa1[:], relu)
                nc.scalar.activation(hid[:, h, FD:N], pb1[:], relu)

            # GEMM2 + relu. Output computed TRANSPOSED (psum [o=128, n=512]:
            # lhsT = w2 o-chunk, rhs = hiddenT n-half) so matmuls stream
            # N=512 — half as many matmuls as the [n, o] mapping and the
            # per-matmul LDWEIGHTS fully hides under the 213ns stream.
            # Stored via strided DMA (128 o-values = 512B contiguous chunks).
            # Accumulated in SBUF: one store per expert (per-tile stores'
            # HWDGE sem-lane reuse couples to in-flight prefetch loads and
            # stalls the relu/psum pipeline mid-GEMM2); last expert stores
            # per tile instead to shorten the kernel tail.
            o_sb = op.tile([P, 2, NCH, FD], f32, tag="o")
            last_e = e == E_PER - 1
            for nh in range(NCH):
                for oc in range(2):
                    po = ps2.tile([P, FD], f32, tag="ps2")
                    for k in range(HT):
                        nc.tensor.matmul(
                            po[:], w2_sb[:, k, bass.ts(oc, P)],
                            hid[:, k, bass.ds(nh * FD, FD)],
                            start=(k == 0), stop=(k == HT - 1))
                    nc.scalar.activation(o_sb[:, oc, nh, :], po[:], relu)
                    if last_e:
                        nc.scalar.dma_start(
                            out_d[e, bass.ds(oc * P, P), bass.ds(nh * FD, FD)],
                            o_sb[:, oc, nh, :])
            if not last_e:
                for oc in range(2):
                    nc.scalar.dma_start(out_d[e, bass.ds(oc * P, P), :],
                                        o_sb[:, oc])

    nc.compile()
    _CACHE["nc"] = nc
    _CACHE["names"] = ("xt", "w1t", "w2t", "out")
    return nc, _CACHE["names"]


def _prep_inputs(x: np.ndarray, w1: np.ndarray, w2: np.ndarray):
    """Shard across cores + cast bf16 + pre-tile so all DMAs are contiguous."""
    # xT, partition-major: xt[e, p, d, n] = x[e, n, d*128+p]
    xt = np.ascontiguousarray(
        x.astype(_BF16).transpose(0, 2, 1)      # [E, D_IN, N]
        .reshape(E, DT, P, N).transpose(0, 2, 1, 3))  # [E, P, DT, N]
    # w1 h-tiled, partition-major: w1t[e, h, p, dt*128+c] = w1[e, dt*128+p, h*128+c]
    w1t = np.ascontiguousarray(
        w1.astype(_BF16).reshape(E, DT, P, HT, P)
        .transpose(0, 3, 2, 1, 4).reshape(E, HT, P, DT * P))
    # w2 k-tiled, partition-major: w2t[e, p, k, o] = w2[e, k*128+p, o]
    w2t = np.ascontiguousarray(
        w2.astype(_BF16).reshape(E, HT, P, D_OUT).transpose(0, 2, 1, 3))

    in_maps = []
    for c in range(NCORES):
        sl = slice(c * E_PER, (c + 1) * E_PER)
        in_maps.append({"xt": xt[sl], "w1t": w1t[sl], "w2t": w2t[sl]})
    return in_maps


def run(x, w1, w2, trace=False, **trace_kwargs):
    """Run on 8 cores; returns (full_out, BassKernelResults)."""
    from concourse.bass_utils import run_bass_kernel_spmd

    nc, _ = _build_program()
    in_maps = _prep_inputs(np.asarray(x), np.asarray(w1), np.asarray(w2))
    res = run_bass_kernel_spmd(nc, in_maps, list(range(NCORES)), trace=trace,
                               **trace_kwargs)
    out_t = np.concatenate([res.results[c]["out"] for c in range(NCORES)],
                           axis=0)  # [E, D_OUT, N]
    out = np.ascontiguousarray(out_t.transpose(0, 2, 1))
    return out, res


def _run_in_subprocess(x, w1, w2):
    """Fallback: execute in a fresh interpreter. The NeuronCores are
    occasionally left wedged (NRT_EXEC_UNIT_UNRECOVERABLE on the next
    execute); a fresh process + axon client re-init recovers."""
    import pickle
    import subprocess
    import sys
    import tempfile

    with tempfile.TemporaryDirectory() as td:
        in_p = f"{td}/in.pkl"
        out_p = f"{td}/out.npy"
        with open(in_p, "wb") as f:
            pickle.dump({"x": x, "w1": w1, "w2": w2}, f, protocol=4)
        subprocess.run([sys.executable, __file__, "--subproc", in_p, out_p],
                       check=True, timeout=1200)
        return np.load(out_p)


def kernel(x: np.ndarray, w1: np.ndarray, w2: np.ndarray) -> np.ndarray:
    try:
        out, _ = run(x, w1, w2, trace=False)
        return out
    except Exception:
        pass
    for attempt in range(3):
        try:
            return _run_in_subprocess(x, w1, w2)
        except Exception:
            if attempt == 2:
                raise
    raise RuntimeError("unreachable")


if __name__ == "__main__":
    import pickle
    import sys

    if len(sys.argv) == 4 and sys.argv[1] == "--subproc":
        with open(sys.argv[2], "rb") as f:
            data = pickle.load(f)
        out, _ = run(data["x"], data["w1"], data["w2"], trace=False)
        np.save(sys.argv[3], out)



# revision 2
# speedup vs baseline: 1.1224x; 1.1224x over previous
"""Trainium2 Bass kernel for per-expert MLP (MoE experts, expert-parallel).

Computes out = relu(relu(x @ w1) @ w2) per expert.
  x:  [E=32, N=1024, D_IN=3072] f32
  w1: [E, D_IN, D_H=1024] f32
  w2: [E, D_H, D_OUT=256] f32
  out:[E, N, D_OUT] f32

Sharding: expert dim E=32 split across 8 cores (4 experts/core), no
communication. Host pre-casts and pre-tiles layouts so every DMA is a
plain partition-major copy and no on-chip transposes are needed.

Precision scheme (error budget rel_l2 < 2e-2):
  - GEMM1 K dim (3072 = 24 d-tiles of 128) split: first NF8=6 d-tiles
    use fp8 e4m3 operands via DoubleRow matmuls (2 k-tiles per matmul
    at 2x rate: 512 cyc/pair instead of 1024), remaining 18 in fp16.
  - Everything else fp16 (same cost as bf16, 8x less quantization
    noise): measured rel_l2 ~1.9e-2, dominated by the fp8 tiles
    (per-pair err 1.096e-2, total sqrt(3)*1.096e-2).

GEMM1 computes hiddenT (h on partitions) directly:
  hiddenT[h, n] = sum_d w1[d, h] * x[n, d]
  lhsT = w1 tile [d(128 part), h(128 cols)]   (stationary)
  rhs  = xT tile [d(128 part), n(512 free)]   (moving)
GEMM2 then has contraction dim h already on partitions:
  outT[o, n] = sum_h w2[h, o] * hiddenT[h, n]
The output is stored transposed ([E, D_OUT, N]) for contiguous DMA and
un-transposed on the host during gather.
"""

import numpy as np
import ml_dtypes

E, N, D_IN, D_H, D_OUT = 32, 1024, 3072, 1024, 256
NCORES = 8
E_PER = E // NCORES  # 4 experts per core
P = 128
DT = D_IN // P  # 24 k-tiles for GEMM1
NF8 = 6         # leading k-tiles in fp8 e4m3 (must be even: DoubleRow pairs)
NPR = NF8 // 2  # DoubleRow pairs
DBF = DT - NF8  # fp16 k-tiles
HT = D_H // P   # 8 h-tiles
NT = N // P     # 8 n-tiles
FD = 512        # matmul free dim (one PSUM bank of f32)
NCH = N // FD   # 2 n-chunks in GEMM1

_F16 = np.float16
_F8 = ml_dtypes.float8_e4m3
_CACHE = {}


def _build_program():
    """Build + compile the per-core Bass program (same program on all cores)."""
    if "nc" in _CACHE:
        return _CACHE["nc"], _CACHE["names"]

    from contextlib import ExitStack

    import concourse.bass as bass
    import concourse.tile as tile
    from concourse import bacc, mybir

    f16 = mybir.dt.float16
    f8 = mybir.dt.float8e4
    f32 = mybir.dt.float32
    DR = mybir.MatmulPerfMode.DoubleRow

    nc = bacc.Bacc("TRN2", target_bir_lowering=False, debug=False,
                   enable_asserts=False)

    # Per-core DRAM I/O (host-prepped layouts, see kernel() below).
    x8_d = nc.dram_tensor("x8t", [E_PER, P, NF8, N], f8,
                          kind="ExternalInput").ap()
    x_d = nc.dram_tensor("xt", [E_PER, P, DBF, N], f16,
                         kind="ExternalInput").ap()
    w18_d = nc.dram_tensor("w18t", [E_PER, HT, P, NF8 * P], f8,
                           kind="ExternalInput").ap()
    w1_d = nc.dram_tensor("w1t", [E_PER, HT, P, DBF * P], f16,
                          kind="ExternalInput").ap()
    w2_d = nc.dram_tensor("w2t", [E_PER, P, HT, D_OUT], f16,
                          kind="ExternalInput").ap()
    # Output stored transposed ([o, n] per expert): GEMM2 computes psum
    # [o=128, n=512] tiles, and this layout makes the store DMA fully
    # contiguous per partition. The host un-transposes after gather.
    out_d = nc.dram_tensor("out", [E_PER, D_OUT, N], f32,
                           kind="ExternalOutput").ap()

    relu = mybir.ActivationFunctionType.Relu

    with tile.TileContext(nc) as tc, ExitStack() as ctx:
        xp8 = ctx.enter_context(tc.tile_pool(name="x8", bufs=2))
        xp = ctx.enter_context(tc.tile_pool(name="x", bufs=2))
        w1p8 = ctx.enter_context(tc.tile_pool(name="w18", bufs=4))
        w1p = ctx.enter_context(tc.tile_pool(name="w1", bufs=4))
        w2p = ctx.enter_context(tc.tile_pool(name="w2", bufs=2))
        hp = ctx.enter_context(tc.tile_pool(name="hid", bufs=2))
        op = ctx.enter_context(tc.tile_pool(name="o", bufs=2))
        wmp = ctx.enter_context(tc.tile_pool(name="warm", bufs=1))
        ps1 = ctx.enter_context(tc.tile_pool(name="ps1", bufs=6, space="PSUM"))
        ps2 = ctx.enter_context(tc.tile_pool(name="ps2", bufs=2, space="PSUM"))

        # PE warm-up: dummy matmuls with no data deps fill the initial DMA
        # wait so the HAM clock-gate is at 8/8 (2.4 GHz) when real matmuls
        # start (the un-throttle needs ~3.4us of sustained PE activity).
        # One accumulation group: no per-matmul PSUM WAW serialization.
        NWARM = 18
        warm = wmp.tile([P, FD], f16, tag="warm")
        nc.vector.memset(warm[:], 0.0)
        pw = ps2.tile([P, FD], f32, tag="ps2", name="pw")
        for i in range(NWARM):
            nc.tensor.matmul(pw[:], warm[:, 0:P], warm[:],
                             start=(i == 0), stop=(i == NWARM - 1))

        for e in range(E_PER):
            # Load order on the sync HWDGE ring (FIFO per engine): fp8
            # operands first (small + feed the leading DoubleRow matmuls),
            # then w1 h0/h1 and the x fp16 stream; remaining w1 + w2 after.
            w18_tiles = []
            w1_tiles = []
            x8_sb = xp8.tile([P, NF8, N], f8, tag="x8")
            x_sb = xp.tile([P, DBF, N], f16, tag="x")
            w18a = w1p8.tile([P, NF8, P], f8, tag="w18")
            w18b = w1p8.tile([P, NF8, P], f8, tag="w18")
            w1_sb = w1p.tile([P, DBF * P], f16, tag="w1")
            if e == 0:
                # Pace the first-expert ramp: the whole e0 ramp is
                # DMA-bandwidth-bound, so start compute ASAP. fp8 tiles
                # lead (half bytes); w1 fp16 h0/h1 chunked so GEMM1's
                # first fp16 matmuls only wait on a ~192KB transfer.
                C6 = 6 * P
                nc.sync.dma_start(w18a[:], w18_d[e, 0])
                w18_tiles.append(w18a)
                nc.sync.dma_start(w18b[:], w18_d[e, 1])
                w18_tiles.append(w18b)
                nc.sync.dma_start(x8_sb[:, 0:2, :], x8_d[e, :, 0:2, :])
                nc.sync.dma_start(x8_sb[:, 2:4, :], x8_d[e, :, 2:4, :])
                nc.sync.dma_start(x8_sb[:, 4:6, :], x8_d[e, :, 4:6, :])
                w1b_sb = w1p.tile([P, DBF * P], f16, tag="w1")
                nc.sync.dma_start(w1_sb[:, 0:C6], w1_d[e, 0, :, 0:C6])
                w1_tiles.append(w1_sb)
                nc.sync.dma_start(x_sb[:, 0, :], x_d[e, :, 0, :])
                nc.sync.dma_start(w1b_sb[:, 0:C6], w1_d[e, 1, :, 0:C6])
                w1_tiles.append(w1b_sb)
                nc.sync.dma_start(x_sb[:, 1, :], x_d[e, :, 1, :])
                nc.sync.dma_start(x_sb[:, 2, :], x_d[e, :, 2, :])
                nc.sync.dma_start(w1_sb[:, C6: 2 * C6], w1_d[e, 0, :, C6: 2 * C6])
                nc.sync.dma_start(w1b_sb[:, C6: 2 * C6], w1_d[e, 1, :, C6: 2 * C6])
                nc.sync.dma_start(x_sb[:, 3, :], x_d[e, :, 3, :])
                nc.sync.dma_start(x_sb[:, 4, :], x_d[e, :, 4, :])
                nc.sync.dma_start(w1_sb[:, 2 * C6: DBF * P],
                                  w1_d[e, 0, :, 2 * C6: DBF * P])
                nc.sync.dma_start(w1b_sb[:, 2 * C6: DBF * P],
                                  w1_d[e, 1, :, 2 * C6: DBF * P])
                for d in range(5, DBF):
                    nc.sync.dma_start(x_sb[:, d, :], x_d[e, :, d, :])
            else:
                # prefetched during previous expert: coarse chunks to limit
                # HWDGE sem-lane churn (8 lanes shared across all queues)
                nc.sync.dma_start(w18a[:], w18_d[e, 0])
                w18_tiles.append(w18a)
                nc.sync.dma_start(w1_sb[:], w1_d[e, 0])
                w1_tiles.append(w1_sb)
                nc.sync.dma_start(x8_sb[:], x8_d[e])
                nc.sync.dma_start(x_sb[:, 0, :], x_d[e, :, 0, :])
                nc.sync.dma_start(w18b[:], w18_d[e, 1])
                w18_tiles.append(w18b)
                w1_sb = w1p.tile([P, DBF * P], f16, tag="w1")
                nc.sync.dma_start(w1_sb[:], w1_d[e, 1])
                w1_tiles.append(w1_sb)
                for d in range(1, 5):
                    nc.sync.dma_start(x_sb[:, d, :], x_d[e, :, d, :])
                for i in range(5, DBF, 4):
                    j = min(i + 4, DBF)
                    nc.sync.dma_start(x_sb[:, i:j, :], x_d[e, :, i:j, :])
            for h in range(2, HT):
                w18_sb = w1p8.tile([P, NF8, P], f8, tag="w18")
                nc.sync.dma_start(w18_sb[:], w18_d[e, h])
                w18_tiles.append(w18_sb)
                w1_sb = w1p.tile([P, DBF * P], f16, tag="w1")
                nc.sync.dma_start(w1_sb[:], w1_d[e, h])
                w1_tiles.append(w1_sb)
            w2_sb = w2p.tile([P, HT, D_OUT], f16, tag="w2")
            nc.sync.dma_start(w2_sb[:], w2_d[e])

            hid = hp.tile([P, HT, N], f16, tag="hid")

            # GEMM1 + relu -> hiddenT (fp16). h0 and h1 are interleaved in
            # one d-pass: each arriving x tile feeds 4 matmuls, so the
            # DMA-paced first-expert ramp consumes x at ~arrival rate
            # instead of stalling h0 on the tail of the x stream. fp8
            # DoubleRow pairs lead (their operands arrive first on e0).
            pa = [ps1.tile([P, FD], f32, tag="ps1", name=f"pa{i}")
                  for i in range(2)]
            pb = [ps1.tile([P, FD], f32, tag="ps1", name=f"pb{i}")
                  for i in range(2)]
            for dp in range(NPR):
                s = slice(2 * dp, 2 * dp + 2)
                for hh in range(2):
                    lhsT8 = w18_tiles[hh][:, s, :]
                    nc.tensor.matmul(pa[hh][:], lhsT8, x8_sb[:, s, 0:FD],
                                     start=(dp == 0), stop=False,
                                     perf_mode=DR)
                    nc.tensor.matmul(pb[hh][:], lhsT8, x8_sb[:, s, FD:N],
                                     start=(dp == 0), stop=False,
                                     perf_mode=DR)
            for d in range(DBF):
                for hh in range(2):
                    lhsT = w1_tiles[hh][:, bass.ts(d, P)]
                    nc.tensor.matmul(pa[hh][:], lhsT, x_sb[:, d, 0:FD],
                                     start=False, stop=(d == DBF - 1))
                    nc.tensor.matmul(pb[hh][:], lhsT, x_sb[:, d, FD:N],
                                     start=False, stop=(d == DBF - 1))
            for hh in range(2):
                nc.scalar.activation(hid[:, hh, 0:FD], pa[hh][:], relu)
                nc.scalar.activation(hid[:, hh, FD:N], pb[hh][:], relu)
            for h in range(2, HT):
                w18_sb = w18_tiles[h]
                w1_sb = w1_tiles[h]
                pa1 = ps1.tile([P, FD], f32, tag="ps1")
                pb1 = ps1.tile([P, FD], f32, tag="ps1")
                for dp in range(NPR):
                    s = slice(2 * dp, 2 * dp + 2)
                    lhsT8 = w18_sb[:, s, :]
                    nc.tensor.matmul(pa1[:], lhsT8, x8_sb[:, s, 0:FD],
                                     start=(dp == 0), stop=False,
                                     perf_mode=DR)
                    nc.tensor.matmul(pb1[:], lhsT8, x8_sb[:, s, FD:N],
                                     start=(dp == 0), stop=False,
                                     perf_mode=DR)
                for d in range(DBF):
                    lhsT = w1_sb[:, bass.ts(d, P)]
                    nc.tensor.matmul(pa1[:], lhsT, x_sb[:, d, 0:FD],
                                     start=False, stop=(d == DBF - 1))
                    nc.tensor.matmul(pb1[:], lhsT, x_sb[:, d, FD:N],
                                     start=False, stop=(d == DBF - 1))
                nc.scalar.activation(hid[:, h, 0:FD], pa1[:], relu)
                nc.scalar.activation(hid[:, h, FD:N], pb1[:], relu)

            # GEMM2 + relu. Output computed TRANSPOSED (psum [o=128, n=512]:
            # lhsT = w2 o-chunk, rhs = hiddenT n-half) so matmuls stream
            # N=512 — half as many matmuls as the [n, o] mapping and the
            # per-matmul LDWEIGHTS fully hides under the 213ns stream.
            # Stored via strided DMA (128 o-values = 512B contiguous chunks).
            # Accumulated in SBUF: one store per expert (per-tile stores'
            # HWDGE sem-lane reuse couples to in-flight prefetch loads and
            # stalls the relu/psum pipeline mid-GEMM2); last expert stores
            # per tile instead to shorten the kernel tail.
            o_sb = op.tile([P, 2, NCH, FD], f32, tag="o")
            last_e = e == E_PER - 1
            for nh in range(NCH):
                for oc in range(2):
                    po = ps2.tile([P, FD], f32, tag="ps2")
                    for k in range(HT):
                        nc.tensor.matmul(
                            po[:], w2_sb[:, k, bass.ts(oc, P)],
                            hid[:, k, bass.ds(nh * FD, FD)],
                            start=(k == 0), stop=(k == HT - 1))
                    nc.scalar.activation(o_sb[:, oc, nh, :], po[:], relu)
                    if last_e:
                        nc.scalar.dma_start(
                            out_d[e, bass.ds(oc * P, P), bass.ds(nh * FD, FD)],
                            o_sb[:, oc, nh, :])
            if not last_e:
                for oc in range(2):
                    nc.scalar.dma_start(out_d[e, bass.ds(oc * P, P), :],
                                        o_sb[:, oc])

    nc.compile()
    _CACHE["nc"] = nc
    _CACHE["names"] = ("x8t", "xt", "w18t", "w1t", "w2t", "out")
    return nc, _CACHE["names"]


def _prep_inputs(x: np.ndarray, w1: np.ndarray, w2: np.ndarray):
    """Shard across cores + cast + pre-tile so all DMAs are contiguous."""
    # xT, partition-major: xt[e, p, d, n] = x[e, n, d*128+p]
    xt = (x.astype(_F16).transpose(0, 2, 1)       # [E, D_IN, N]
          .reshape(E, DT, P, N).transpose(0, 2, 1, 3))  # [E, P, DT, N]
    x8t = np.ascontiguousarray(xt[:, :, 0:NF8, :]).astype(_F8)
    xbt = np.ascontiguousarray(xt[:, :, NF8:, :])
    # w1 h-tiled, partition-major: [E, HT, P, DT, P] with
    # w1t[e, h, p, dt, c] = w1[e, dt*128+p, h*128+c]
    w1t = (w1.astype(_F16).reshape(E, DT, P, HT, P)
           .transpose(0, 3, 2, 1, 4))
    w18t = np.ascontiguousarray(
        w1t[:, :, :, 0:NF8, :]).reshape(E, HT, P, NF8 * P).astype(_F8)
    w1bt = np.ascontiguousarray(
        w1t[:, :, :, NF8:, :]).reshape(E, HT, P, DBF * P)
    # w2 k-tiled, partition-major: w2t[e, p, k, o] = w2[e, k*128+p, o]
    w2t = np.ascontiguousarray(
        w2.astype(_F16).reshape(E, HT, P, D_OUT).transpose(0, 2, 1, 3))

    in_maps = []
    for c in range(NCORES):
        sl = slice(c * E_PER, (c + 1) * E_PER)
        in_maps.append({"x8t": x8t[sl], "xt": xbt[sl], "w18t": w18t[sl],
                        "w1t": w1bt[sl], "w2t": w2t[sl]})
    return in_maps


def run(x, w1, w2, trace=False, **trace_kwargs):
    """Run on 8 cores; returns (full_out, BassKernelResults)."""
    from concourse.bass_utils import run_bass_kernel_spmd

    nc, _ = _build_program()
    in_maps = _prep_inputs(np.asarray(x), np.asarray(w1), np.asarray(w2))
    res = run_bass_kernel_spmd(nc, in_maps, list(range(NCORES)), trace=trace,
                               **trace_kwargs)
    out_t = np.concatenate([res.results[c]["out"] for c in range(NCORES)],
                           axis=0)  # [E, D_OUT, N]
    out = np.ascontiguousarray(out_t.transpose(0, 2, 1))
    return out, res


def _run_in_subprocess(x, w1, w2):
    """Fallback: execute in a fresh interpreter. The NeuronCores are
    occasionally left wedged (NRT_EXEC_UNIT_UNRECOVERABLE on the next
    execute); a fresh process + axon client re-init recovers."""
    import pickle
    import subprocess
    import sys
    import tempfile

    with tempfile.TemporaryDirectory() as td:
        in_p = f"{td}/in.pkl"
        out_p = f"{td}/out.npy"
        with open(in_p, "wb") as f:
            pickle.dump({"x": x, "w1": w1, "w2": w2}, f, protocol=4)
        subprocess.run([sys.executable, __file__, "--subproc", in_p, out_p],
                       check=True, timeout=1200)
        return np.load(out_p)


def kernel(x: np.ndarray, w1: np.ndarray, w2: np.ndarray) -> np.ndarray:
    try:
        out, _ = run(x, w1, w2, trace=False)
        return out
    except Exception:
        pass
    for attempt in range(3):
        try:
            return _run_in_subprocess(x, w1, w2)
        except Exception:
            if attempt == 2:
                raise
    raise RuntimeError("unreachable")


if __name__ == "__main__":
    import pickle
    import sys

    if len(sys.argv) == 4 and sys.argv[1] == "--subproc":
        with open(sys.argv[2], "rb") as f:
            data = pickle.load(f)
        out, _ = run(data["x"], data["w1"], data["w2"], trace=False)
        np.save(sys.argv[3], out)
